# revision 17
# baseline (speedup 1.0000x reference)
"""GCN layer (fc + gather/scatter-sum) on 8 trn2 NeuronCores.

v2: FC-first h-table + compact SWDGE gather.

  h = feature @ W + b   (built on-device, per core, into a DRAM table of
                         bf16 rows padded to 128 cols = 256B — the minimum
                         SWDGE gather element)
  out = segment_sum(take(h, src), dst)   (dma_gather of 256B rows + one-hot
                         matmul aggregation per 128-dst block)

The desc-gen ucode (one Q7 pair, ~3.5ns/desc serialized on GpSimd) is the
wall, so descriptors are minimized: edges are packed COMPACTLY per call
(no per-(block,group) tile rounding) and per-core trailing -1 indices let
the ucode trim to each core's actual edge count. Tiles may straddle block
boundaries; the one-hot dst matrix is built from host-shipped per-(block,
tile) columns (foreign/pad slots = -1, so is_equal never matches).
Block PSUM accumulators live across all 4 source-window calls; blocks are
processed in two phases (52+46) to fit PSUM. The h-table is a DRAM tile,
so Tile's subtile deps order window builds before their gathers, letting
phase-0 FC overlap the first window's gather stream.
"""

import os
import numpy as np
import ml_dtypes

import concourse.bass as bass
import concourse.bacc as bacc
import concourse.mybir as mybir
from concourse import tile
from concourse import library_config

P = 128
NCORES = 8


def _patch_tile_exit():
    """The walrus build in this container rejects two constructs Tile emits
    at TileContext exit: a Drain carrying more than one sync wait ("Too many
    sync wait commands") and the sem_clear InstISA ("ISA wrong length").
    Replace the exit sequence with equivalent one-wait-per-Drain chains and
    skip the semaphore clears (fine for single-execution NEFFs)."""
    import bass_rust
    from concourse.vector_clock import ScopedClock

    def _drain_and_barrier(self, tick_clock, wait_clock):
        drain_inst = self.nc.sync.drain()
        wait_clock.add_sem_waits(
            drain_inst.ins, ScopedClock({None: tick_clock.global_clock})
        )
        si = drain_inst.ins.sync_info
        if si is not None and len(si.on_wait) > 1:
            waits = list(si.on_wait)
            drain_inst.ins.sync_info = bass_rust.SyncInfo(
                on_wait=waits[:1], on_update=list(si.on_update))
            for w in waits[1:]:
                extra = self.nc.sync.drain()
                extra.ins.sync_info = bass_rust.SyncInfo(
                    on_wait=[w], on_update=[])
        self.nc.all_engine_barrier()
        popped = self.nc._tile_sem_poison_stack.pop()
        assert popped is self._sem_poison
        self.nc.all_engine_barrier()

    tile.TileContext._drain_and_barrier = _drain_and_barrier


_patch_tile_exit()


class Cfg:
    def __init__(self, n_nodes, d_in, d_out, ncores, group_shift, sb_blocks,
                 psum_blocks):
        self.N = n_nodes
        self.D = d_in
        self.DO = d_out
        self.ncores = ncores
        self.gshift = group_shift          # src window = [g << gshift, (g+1) << gshift)
        self.gsize = 1 << group_shift
        self.ngroups = (n_nodes + self.gsize - 1) >> group_shift
        self.npc = n_nodes // ncores       # nodes per core
        self.nblk = (self.npc + P - 1) // P
        self.sb = sb_blocks                # blocks per superbatch (= per call)
        self.psb = psum_blocks             # max concurrent block accumulators
        self.ntab = ((n_nodes + P - 1) // P) * P  # padded table rows


FULL_CFG = Cfg(n_nodes=100000, d_in=256, d_out=64, ncores=8, group_shift=15,
               sb_blocks=13, psum_blocks=52)


def _prep_host(feature, W, b, src, dst, cfg):
    """Shard + sort edges; build compact per-call slot layout and the
    per-(block, tile) one-hot dst columns."""
    N, npc, nblk, ng = cfg.N, cfg.npc, cfg.nblk, cfg.ngroups
    src = np.asarray(src, dtype=np.int64)
    dst = np.asarray(dst, dtype=np.int64)

    nsb = (nblk + cfg.sb - 1) // cfg.sb
    supers = [range(s * cfg.sb, min((s + 1) * cfg.sb, nblk)) for s in range(nsb)]

    # per-core sorted edge lists per (super, group, block)
    per_core = []   # m -> {(s, g, k): (es, ed_rel)}
    counts = np.zeros((cfg.ncores, nsb, ng, nblk), dtype=np.int64)
    for m in range(cfg.ncores):
        lo, hi = m * npc, (m + 1) * npc
        mask = (dst >= lo) & (dst < hi)
        es = src[mask]
        ed = dst[mask] - lo
        blk = ed >> 7
        grp = es >> cfg.gshift
        sup = blk // cfg.sb
        order = np.lexsort((es, blk, grp, sup))
        es, ed, blk, grp, sup = (a[order] for a in (es, ed, blk, grp, sup))
        np.add.at(counts[m], (sup, grp, blk), 1)
        key = sup * (ng * nblk) + grp * nblk + blk
        bounds = np.searchsorted(key, np.arange(nsb * ng * nblk + 1))
        d = {}
        for s in range(nsb):
            for g in range(ng):
                for k in supers[s]:
                    a0 = bounds[s * ng * nblk + g * nblk + k]
                    a1 = bounds[s * ng * nblk + g * nblk + k + 1]
                    if a1 > a0:
                        d[(s, g, k)] = (es[a0:a1], ed[a0:a1])
        per_core.append(d)

    # call order: g -> s.  Compact slot layout per call; shared call
    # budget = roundup(max_m total, P).  Shared per-(k) tile spans =
    # union over cores of [start/P, ceil(end/P)).
    call_list = []          # (s, g) in call order
    call_budget = {}        # (s, g) -> (slot_offset, B)
    spans = {}              # (s, g, k) -> (t0, t1) tile span within call
    core_starts = {}        # (m, s, g, k) -> slot start within call
    pos = 0
    for g in range(ng):
        for s in range(nsb):
            tot_m = counts[:, s, g, :].sum(axis=1)
            B = int(np.ceil(max(int(tot_m.max()), 1) / P)) * P
            call_list.append((s, g))
            call_budget[(s, g)] = (pos, B)
            t0s = {k: 10 ** 9 for k in supers[s]}
            t1s = {k: -1 for k in supers[s]}
            for m in range(cfg.ncores):
                off = 0
                for k in supers[s]:
                    n = int(counts[m, s, g, k])
                    core_starts[(m, s, g, k)] = off
                    if n > 0:
                        t0s[k] = min(t0s[k], off // P)
                        t1s[k] = max(t1s[k], (off + n - 1) // P)
                    off += n
            for k in supers[s]:
                if t1s[k] >= 0:
                    spans[(s, g, k)] = (t0s[k], t1s[k] + 1)
            pos += B
    tot = pos
    assert tot % P == 0

    # column layout: per call, for k in super, for t in span(k): one column
    col_index = {}          # (s, g, k, t) -> global column index
    call_cols = {}          # (s, g) -> (col0, ncols)
    cpos = 0
    for (s, g) in call_list:
        c0 = cpos
        for k in supers[s]:
            if (s, g, k) not in spans:
                continue
            t0, t1 = spans[(s, g, k)]
            for t in range(t0, t1):
                col_index[(s, g, k, t)] = cpos
                cpos += 1
        call_cols[(s, g)] = (c0, cpos - c0)
    totcols = max(cpos, 1)

    # per-(k) global matmul sequence (for start/stop flags)
    kseq = {k: [] for k in range(nblk)}
    for (s, g) in call_list:
        for k in supers[s]:
            if (s, g, k) not in spans:
                continue
            t0, t1 = spans[(s, g, k)]
            for t in range(t0, t1):
                kseq[k].append((s, g, t, col_index[(s, g, k, t)]))
    for k in range(nblk):
        assert kseq[k], f"block {k} has no edges on some core"

    # host arrays
    ftab = np.ascontiguousarray(feature.astype(ml_dtypes.bfloat16))
    ntiles = cfg.ntab // P
    ftt = np.zeros((ntiles, cfg.D // P, P, P), dtype=ml_dtypes.bfloat16)
    for j in range(ntiles):
        rows = min(P, N - j * P)
        blkf = ftab[j * P:j * P + rows, :]          # [rows, D]
        for c in range(cfg.D // P):
            ftt[j, c, :, :rows] = blkf[:, c * P:(c + 1) * P].T
    wmat = np.ascontiguousarray(W.astype(ml_dtypes.bfloat16))
    bbc = np.ascontiguousarray(
        np.tile(b.astype(np.float32)[None, :], (P, 1)))
    iota = np.ascontiguousarray(
        np.tile(np.arange(P, dtype=np.float32)[None, :], (P, 1)).astype(
            ml_dtypes.bfloat16))

    # pad slots gather row 0 of the window (a -1/trailing-trim variant
    # crashed the device; zero-pad costs ~2% extra descriptors)
    in_maps = []
    for m in range(cfg.ncores):
        d = per_core[m]
        idx_arr = np.zeros(tot, dtype=np.int16)
        dslot = np.full(tot, -1.0, dtype=np.float32)   # dst-local per slot
        kown = np.full(tot, -2, dtype=np.int64)        # owning block per slot
        for (s, g) in call_list:
            off, B = call_budget[(s, g)]
            for k in supers[s]:
                if (s, g, k) not in d:
                    continue
                es, ed = d[(s, g, k)]
                a = core_starts[(m, s, g, k)]
                n = len(es)
                idx_arr[off + a:off + a + n] = (
                    es - (g << cfg.gshift)).astype(np.int16)
                dslot[off + a:off + a + n] = (ed - k * P).astype(np.float32)
                kown[off + a:off + a + n] = k
        # per-(k,t) one-hot columns: partition p of column (s,g,k,t) =
        # dst-local of slot off+t*P+p if that slot belongs to block k else -1
        dcols = np.full((P, totcols), -1.0, dtype=np.float32)
        for (s, g, k, t), col in col_index.items():
            off, B = call_budget[(s, g)]
            sl = slice(off + t * P, off + t * P + P)
            dcols[:, col] = np.where(kown[sl] == k, dslot[sl], -1.0)
        idx16 = np.ascontiguousarray(
            np.tile(idx_arr.reshape(tot // 16, 16).T, (P // 16, 1)))
        dcolst = np.ascontiguousarray(dcols.astype(ml_dtypes.bfloat16))
        in_maps.append({
            "ftt": ftt, "wmat": wmat, "bbc": bbc, "iota": iota,
            "idx16": idx16, "dcolst": dcolst,
        })

    lastcall = {k: (kseq[k][-1][0], kseq[k][-1][1]) for k in range(nblk)}
    meta = dict(call_list=call_list, call_budget=call_budget, spans=spans,
                kseq=kseq, supers=supers, tot=tot, totcols=totcols,
                call_cols=call_cols, col_index=col_index, lastcall=lastcall)
    return in_maps, meta


def _build_program(cfg, meta):
    N, D, DO, nblk, ng = cfg.N, cfg.D, cfg.DO, cfg.nblk, cfg.ngroups
    call_list, call_budget = meta["call_list"], meta["call_budget"]
    spans, kseq, supers = meta["spans"], meta["kseq"], meta["supers"]
    tot, totcols = meta["tot"], meta["totcols"]
    bf16, f32, i16 = mybir.dt.bfloat16, mybir.dt.float32, mybir.dt.int16
    ntiles = cfg.ntab // P
    kchunks = D // P

    nc = bacc.Bacc(None, target_bir_lowering=False, num_swdge_queues=4)
    ftt = nc.dram_tensor("ftt", [ntiles, kchunks, P, P], bf16,
                         kind="ExternalInput")
    wmat = nc.dram_tensor("wmat", [D, DO], bf16, kind="ExternalInput")
    bbc = nc.dram_tensor("bbc", [P, DO], f32, kind="ExternalInput")
    iota = nc.dram_tensor("iota", [P, P], bf16, kind="ExternalInput")
    idx16 = nc.dram_tensor("idx16", [P, tot // 16], i16, kind="ExternalInput")
    dcolst = nc.dram_tensor("dcolst", [P, totcols], bf16, kind="ExternalInput")
    out = nc.dram_tensor("out", [cfg.npc, DO], f32, kind="ExternalOutput")

    # h-table rows padded to 128 cols (256B) for the SWDGE elem minimum
    HC = 2 * DO

    with tile.TileContext(nc) as tc:
        with (
            tc.tile_pool(name="const", bufs=1) as cpool,
            tc.tile_pool(name="dram", bufs=1, space="DRAM") as dpool,
            tc.tile_pool(name="ftp", bufs=3) as ftpool,
            tc.tile_pool(name="hst", bufs=3) as hpool,
            tc.tile_pool(name="gath", bufs=3) as gpool,
            tc.tile_pool(name="amat", bufs=2) as apool,
            tc.tile_pool(name="work", bufs=4) as wpool,
            tc.tile_pool(name="ps0", bufs=2, space="PSUM") as ps0,
            tc.tile_pool(name="psag", bufs=4, space="PSUM") as psag,
        ):
            htab = dpool.tile([cfg.ntab, HC], bf16)
            accum = cpool.tile([P, nblk * DO], f32)
            nc.vector.memset(accum[:], 0.0)

            idxt = cpool.tile([P, tot // 16], i16)
            nc.sync.dma_start(out=idxt[:], in_=idx16[:])
            dct = cpool.tile([P, totcols], bf16)
            nc.sync.dma_start(out=dct[:], in_=dcolst[:])
            iotat = cpool.tile([P, P], bf16)
            nc.sync.dma_start(out=iotat[:], in_=iota[:])
            bbct = cpool.tile([P, DO], f32)
            nc.sync.dma_start(out=bbct[:], in_=bbc[:])
            wts = []
            for c in range(kchunks):
                wt = cpool.tile([P, DO], bf16, tag=f"w{c}")
                nc.sync.dma_start(out=wt[:], in_=wmat[c * P:(c + 1) * P, :])
                wts.append(wt)

            gsz_regs = [nc.alloc_register(mybir.EngineType.Pool, f"gsz{q}")
                        for q in range(4)]

            # ---- phase 0: build h-table, window-interleaved with calls ----
            win_tiles = []
            for g in range(ng):
                j0 = (g << cfg.gshift) // P
                j1 = min(((g + 1) << cfg.gshift), cfg.ntab) // P
                win_tiles.append((j0, j1))

            def build_window(g):
                j0, j1 = win_tiles[g]
                for j in range(j0, j1):
                    ft = ftpool.tile([P, kchunks * P], bf16, tag="ft")
                    for c in range(kchunks):
                        nc.sync.dma_start(out=ft[:, c * P:(c + 1) * P],
                                          in_=ftt[j, c])
                    ph = ps0.tile([P, DO], f32, tag="ph")
                    for c in range(kchunks):
                        nc.tensor.matmul(ph[:], lhsT=ft[:, c * P:(c + 1) * P],
                                         rhs=wts[c][:], start=(c == 0),
                                         stop=(c == kchunks - 1))
                    hs = hpool.tile([P, DO], bf16, tag="hs")
                    nc.vector.tensor_tensor(out=hs[:], in0=ph[:],
                                            in1=bbct[:],
                                            op=mybir.AluOpType.add)
                    # rows are 256B-strided; only the first 64 cols are ever
                    # read (matmul rhs slice), so the right half stays junk
                    nc.sync.dma_start(out=htab[j * P:(j + 1) * P, 0:DO],
                                      in_=hs[:])

            # ---- main: calls in (g, s) order ------------------------------
            B_max = max(b for _, b in call_budget.values())
            for _ in range(3):
                gz = gpool.tile([P, (B_max // P) * HC], bf16, tag="gt",
                                name="gz")
                nc.vector.memset(gz[:], 0.0)

            built = set()
            call_no = 0
            for (s, g) in call_list:
                if g not in built:
                    build_window(g)
                    built.add(g)
                off, B = call_budget[(s, g)]
                glo = g << cfg.gshift
                ghi = min(glo + cfg.gsize, cfg.ntab)
                gt = gpool.tile([P, (B // P) * HC], bf16, tag="gt")
                gt3 = gt[:].rearrange("p (t e) -> p t e", e=HC)
                if os.environ.get("GCN_SKIP_GATHER"):
                    nc.vector.memset(gt[:, 0:1], 0.0)
                else:
                    q = call_no % 4
                    nc.gpsimd.reg_mov(gsz_regs[q], B)
                    nc.gpsimd.dma_gather(
                        out_ap=gt3,
                        in_ap=htab[glo:ghi, :],
                        idxs_ap=idxt[:, off // 16:(off + B) // 16],
                        num_idxs=B,
                        num_idxs_reg=gsz_regs[q],
                        elem_size=HC,
                        single_packet=False,
                        queue_num=q,
                    )
                call_no += 1

                # one-hot matrix for the whole call
                c0, ncols = meta["call_cols"][(s, g)]
                if ncols > 0:
                    ab = apool.tile([P, ncols * P], bf16, tag="ab")
                    d_b = dct[:, c0:c0 + ncols].to_broadcast([P, ncols, P])
                    iap = iotat[:]
                    i_b = bass.AP(iap.tensor, iap.offset,
                                  [iap.ap[0], [0, ncols], iap.ap[1]])
                    nc.vector.tensor_tensor(
                        out=ab[:].rearrange("p (t d) -> p t d", d=P),
                        in0=i_b, in1=d_b, op=mybir.AluOpType.is_equal)

                for k in supers[s]:
                    if (s, g, k) not in spans:
                        continue
                    t0, t1 = spans[(s, g, k)]
                    ps = psag.tile([P, DO], f32, tag="agg", name="aggps")
                    for t in range(t0, t1):
                        col = meta["col_index"][(s, g, k, t)]
                        amat = ab[:, (col - c0) * P:(col - c0 + 1) * P]
                        nc.tensor.matmul(ps[:], lhsT=amat,
                                         rhs=gt3[:, t, 0:DO],
                                         start=(t == t0), stop=(t == t1 - 1))
                    acc_k = accum[:, k * DO:(k + 1) * DO]
                    nc.vector.tensor_tensor(out=acc_k, in0=acc_k, in1=ps[:],
                                            op=mybir.AluOpType.add)
                    if meta["lastcall"][k] == (s, g):
                        rows = min(P, cfg.npc - k * P)
                        nc.sync.dma_start(out=out[k * P:k * P + rows, :],
                                          in_=acc_k[:rows])
    return nc


def _run_spmd(nc, in_maps, trace=False):
    from concourse.bass_utils import run_bass_kernel_spmd
    return run_bass_kernel_spmd(nc, in_maps, list(range(len(in_maps))),
                                trace=trace)


_PROGRAM_CACHE = {}


def gcn_kernel(feature, W, b, src, dst, cfg=FULL_CFG, trace=False):
    in_maps, meta = _prep_host(feature, W, b, src, dst, cfg)
    key = (cfg.N, meta["tot"], meta["totcols"],
           tuple(v for _, v in sorted(meta["call_budget"].items())),
           tuple(sorted(meta["col_index"].keys())))
    nc = _PROGRAM_CACHE.get(key)
    if nc is None:
        nc = _build_program(cfg, meta)
        nc.finalize()
        _PROGRAM_CACHE[key] = nc
    res = _run_spmd(nc, in_maps, trace=trace)
    outs = [res.results[m]["out"] for m in range(cfg.ncores)]
    full = np.concatenate(outs, axis=0).astype(np.float32)
    return full, res


def kernel(**inputs):
    feature = np.asarray(inputs["feature"], dtype=np.float32)
    W = np.asarray(inputs["W"], dtype=np.float32)
    b = np.asarray(inputs["b"], dtype=np.float32)
    src = np.asarray(inputs["src"], dtype=np.int32)
    dst = np.asarray(inputs["dst"], dtype=np.int32)
    full, _ = gcn_kernel(feature, W, b, src, dst, FULL_CFG)
    return full


# revision 22
# speedup vs baseline: 1.4171x; 1.4171x over previous
"""GCN layer (fc + gather/scatter-sum) on 8 trn2 NeuronCores.

v2: FC-first h-table + compact SWDGE gather.

  h = feature @ W + b   (built on-device, per core, into a DRAM table of
                         bf16 rows padded to 128 cols = 256B — the minimum
                         SWDGE gather element)
  out = segment_sum(take(h, src), dst)   (dma_gather of 256B rows + one-hot
                         matmul aggregation per 128-dst block)

The desc-gen ucode (one Q7 pair, ~3.5ns/desc serialized on GpSimd) is the
wall, so descriptors are minimized: edges are packed COMPACTLY per call
(no per-(block,group) tile rounding) and per-core trailing -1 indices let
the ucode trim to each core's actual edge count. Tiles may straddle block
boundaries; the one-hot dst matrix is built from host-shipped per-(block,
tile) columns (foreign/pad slots = -1, so is_equal never matches).
Block PSUM accumulators live across all 4 source-window calls; blocks are
processed in two phases (52+46) to fit PSUM. The h-table is a DRAM tile,
so Tile's subtile deps order window builds before their gathers, letting
phase-0 FC overlap the first window's gather stream.
"""

import os
import numpy as np
import ml_dtypes

import concourse.bass as bass
import concourse.bacc as bacc
import concourse.mybir as mybir
from concourse import tile
from concourse import library_config

P = 128
NCORES = 8


def _patch_tile_exit():
    """The walrus build in this container rejects two constructs Tile emits
    at TileContext exit: a Drain carrying more than one sync wait ("Too many
    sync wait commands") and the sem_clear InstISA ("ISA wrong length").
    Replace the exit sequence with equivalent one-wait-per-Drain chains and
    skip the semaphore clears (fine for single-execution NEFFs)."""
    import bass_rust
    from concourse.vector_clock import ScopedClock

    def _drain_and_barrier(self, tick_clock, wait_clock):
        drain_inst = self.nc.sync.drain()
        wait_clock.add_sem_waits(
            drain_inst.ins, ScopedClock({None: tick_clock.global_clock})
        )
        si = drain_inst.ins.sync_info
        if si is not None and len(si.on_wait) > 1:
            waits = list(si.on_wait)
            drain_inst.ins.sync_info = bass_rust.SyncInfo(
                on_wait=waits[:1], on_update=list(si.on_update))
            for w in waits[1:]:
                extra = self.nc.sync.drain()
                extra.ins.sync_info = bass_rust.SyncInfo(
                    on_wait=[w], on_update=[])
        self.nc.all_engine_barrier()
        popped = self.nc._tile_sem_poison_stack.pop()
        assert popped is self._sem_poison
        self.nc.all_engine_barrier()

    tile.TileContext._drain_and_barrier = _drain_and_barrier


_patch_tile_exit()


class Cfg:
    def __init__(self, n_nodes, d_in, d_out, ncores, group_shift, sb_blocks,
                 psum_blocks):
        self.N = n_nodes
        self.D = d_in
        self.DO = d_out
        self.ncores = ncores
        self.gshift = group_shift          # src window = [g << gshift, (g+1) << gshift)
        self.gsize = 1 << group_shift
        self.ngroups = (n_nodes + self.gsize - 1) >> group_shift
        self.npc = n_nodes // ncores       # nodes per core
        self.nblk = (self.npc + P - 1) // P
        self.sb = sb_blocks                # blocks per superbatch (= per call)
        self.psb = psum_blocks             # max concurrent block accumulators
        self.ntab = ((n_nodes + P - 1) // P) * P  # padded table rows


FULL_CFG = Cfg(n_nodes=100000, d_in=256, d_out=64, ncores=8, group_shift=15,
               sb_blocks=13, psum_blocks=52)


def _prep_host(feature, W, b, src, dst, cfg):
    """Shard + sort edges; build compact per-call slot layout and the
    per-(block, tile) one-hot dst columns."""
    N, npc, nblk, ng = cfg.N, cfg.npc, cfg.nblk, cfg.ngroups
    src = np.asarray(src, dtype=np.int64)
    dst = np.asarray(dst, dtype=np.int64)

    nsb = (nblk + cfg.sb - 1) // cfg.sb
    supers = [range(s * cfg.sb, min((s + 1) * cfg.sb, nblk)) for s in range(nsb)]

    # per-core sorted edge lists per (super, group, block)
    per_core = []   # m -> {(s, g, k): (es, ed_rel)}
    counts = np.zeros((cfg.ncores, nsb, ng, nblk), dtype=np.int64)
    for m in range(cfg.ncores):
        lo, hi = m * npc, (m + 1) * npc
        mask = (dst >= lo) & (dst < hi)
        es = src[mask]
        ed = dst[mask] - lo
        blk = ed >> 7
        grp = es >> cfg.gshift
        sup = blk // cfg.sb
        order = np.lexsort((es, blk, grp, sup))
        es, ed, blk, grp, sup = (a[order] for a in (es, ed, blk, grp, sup))
        np.add.at(counts[m], (sup, grp, blk), 1)
        key = sup * (ng * nblk) + grp * nblk + blk
        bounds = np.searchsorted(key, np.arange(nsb * ng * nblk + 1))
        d = {}
        for s in range(nsb):
            for g in range(ng):
                for k in supers[s]:
                    a0 = bounds[s * ng * nblk + g * nblk + k]
                    a1 = bounds[s * ng * nblk + g * nblk + k + 1]
                    if a1 > a0:
                        d[(s, g, k)] = (es[a0:a1], ed[a0:a1])
        per_core.append(d)

    # call order: g -> s.  Compact slot layout per call; shared call
    # budget = roundup(max_m total, P).  Shared per-(k) tile spans =
    # union over cores of [start/P, ceil(end/P)).
    call_list = []          # (s, g) in call order
    call_budget = {}        # (s, g) -> (slot_offset, B)
    spans = {}              # (s, g, k) -> (t0, t1) tile span within call
    core_starts = {}        # (m, s, g, k) -> slot start within call
    pos = 0
    for g in range(ng):
        for s in range(nsb):
            tot_m = counts[:, s, g, :].sum(axis=1)
            B = int(np.ceil(max(int(tot_m.max()), 1) / P)) * P
            call_list.append((s, g))
            call_budget[(s, g)] = (pos, B)
            t0s = {k: 10 ** 9 for k in supers[s]}
            t1s = {k: -1 for k in supers[s]}
            for m in range(cfg.ncores):
                off = 0
                for k in supers[s]:
                    n = int(counts[m, s, g, k])
                    core_starts[(m, s, g, k)] = off
                    if n > 0:
                        t0s[k] = min(t0s[k], off // P)
                        t1s[k] = max(t1s[k], (off + n - 1) // P)
                    off += n
            for k in supers[s]:
                if t1s[k] >= 0:
                    spans[(s, g, k)] = (t0s[k], t1s[k] + 1)
            pos += B
    tot = pos
    assert tot % P == 0

    # column layout: per call, for k in super, for t in span(k): one column
    col_index = {}          # (s, g, k, t) -> global column index
    call_cols = {}          # (s, g) -> (col0, ncols)
    cpos = 0
    for (s, g) in call_list:
        c0 = cpos
        for k in supers[s]:
            if (s, g, k) not in spans:
                continue
            t0, t1 = spans[(s, g, k)]
            for t in range(t0, t1):
                col_index[(s, g, k, t)] = cpos
                cpos += 1
        call_cols[(s, g)] = (c0, cpos - c0)
    totcols = max(cpos, 1)

    # per-(k) global matmul sequence (for start/stop flags)
    kseq = {k: [] for k in range(nblk)}
    for (s, g) in call_list:
        for k in supers[s]:
            if (s, g, k) not in spans:
                continue
            t0, t1 = spans[(s, g, k)]
            for t in range(t0, t1):
                kseq[k].append((s, g, t, col_index[(s, g, k, t)]))
    for k in range(nblk):
        assert kseq[k], f"block {k} has no edges on some core"

    # host arrays
    ftab = np.ascontiguousarray(feature.astype(ml_dtypes.bfloat16))
    ntiles = cfg.ntab // P
    kchunks = cfg.D // P
    # partition-major FC operand: ftt[p, ((j*kchunks+c)*P)+n] =
    #   feature[j*P+n, c*P+p]  (so ft[:, (j*kchunks+c)*P:+P] is lhsT)
    fpad = np.zeros((cfg.ntab, cfg.D), dtype=ml_dtypes.bfloat16)
    fpad[:N] = ftab
    ftt = np.ascontiguousarray(
        fpad.reshape(ntiles, P, kchunks, P).transpose(3, 0, 2, 1)
        .reshape(P, ntiles * kchunks * P))
    wmat = np.ascontiguousarray(W.astype(ml_dtypes.bfloat16))
    bbc = np.ascontiguousarray(
        np.tile(b.astype(np.float32)[None, :], (P, 1)))
    iota = np.ascontiguousarray(
        np.tile(np.arange(P, dtype=np.float32)[None, :], (P, 1)).astype(
            ml_dtypes.bfloat16))

    # pad slots gather row 0 of the window (a -1/trailing-trim variant
    # crashed the device; zero-pad costs ~2% extra descriptors)
    in_maps = []
    for m in range(cfg.ncores):
        d = per_core[m]
        idx_arr = np.zeros(tot, dtype=np.int16)
        dslot = np.full(tot, -1.0, dtype=np.float32)   # dst-local per slot
        kown = np.full(tot, -2, dtype=np.int64)        # owning block per slot
        for (s, g) in call_list:
            off, B = call_budget[(s, g)]
            for k in supers[s]:
                if (s, g, k) not in d:
                    continue
                es, ed = d[(s, g, k)]
                a = core_starts[(m, s, g, k)]
                n = len(es)
                idx_arr[off + a:off + a + n] = (
                    es - (g << cfg.gshift)).astype(np.int16)
                dslot[off + a:off + a + n] = (ed - k * P).astype(np.float32)
                kown[off + a:off + a + n] = k
        # per-(k,t) one-hot columns: partition p of column (s,g,k,t) =
        # dst-local of slot off+t*P+p if that slot belongs to block k else -1
        dcols = np.full((P, totcols), -1.0, dtype=np.float32)
        for (s, g, k, t), col in col_index.items():
            off, B = call_budget[(s, g)]
            sl = slice(off + t * P, off + t * P + P)
            dcols[:, col] = np.where(kown[sl] == k, dslot[sl], -1.0)
        idx16 = np.ascontiguousarray(
            np.tile(idx_arr.reshape(tot // 16, 16).T, (P // 16, 1)))
        dcolst = np.ascontiguousarray(dcols.astype(ml_dtypes.bfloat16))
        in_maps.append({
            "ftt": ftt, "wmat": wmat, "bbc": bbc, "iota": iota,
            "idx16": idx16, "dcolst": dcolst,
        })

    lastcall = {k: (kseq[k][-1][0], kseq[k][-1][1]) for k in range(nblk)}
    meta = dict(call_list=call_list, call_budget=call_budget, spans=spans,
                kseq=kseq, supers=supers, tot=tot, totcols=totcols,
                call_cols=call_cols, col_index=col_index, lastcall=lastcall)
    return in_maps, meta


def _build_program(cfg, meta):
    N, D, DO, nblk, ng = cfg.N, cfg.D, cfg.DO, cfg.nblk, cfg.ngroups
    call_list, call_budget = meta["call_list"], meta["call_budget"]
    spans, kseq, supers = meta["spans"], meta["kseq"], meta["supers"]
    tot, totcols = meta["tot"], meta["totcols"]
    bf16, f32, i16 = mybir.dt.bfloat16, mybir.dt.float32, mybir.dt.int16
    ntiles = cfg.ntab // P
    kchunks = D // P

    nc = bacc.Bacc(None, target_bir_lowering=False, num_swdge_queues=4)
    ftt = nc.dram_tensor("ftt", [P, ntiles * kchunks * P], bf16,
                         kind="ExternalInput")
    wmat = nc.dram_tensor("wmat", [D, DO], bf16, kind="ExternalInput")
    bbc = nc.dram_tensor("bbc", [P, DO], f32, kind="ExternalInput")
    iota = nc.dram_tensor("iota", [P, P], bf16, kind="ExternalInput")
    idx16 = nc.dram_tensor("idx16", [P, tot // 16], i16, kind="ExternalInput")
    dcolst = nc.dram_tensor("dcolst", [P, totcols], bf16, kind="ExternalInput")
    out = nc.dram_tensor("out", [cfg.npc, DO], f32, kind="ExternalOutput")

    # h-table rows padded to 128 cols (256B) for the SWDGE elem minimum
    HC = 2 * DO

    with tile.TileContext(nc) as tc:
        with (
            tc.tile_pool(name="const", bufs=1) as cpool,
            tc.tile_pool(name="dram", bufs=1, space="DRAM") as dpool,
            tc.tile_pool(name="ftp", bufs=3) as ftpool,
            tc.tile_pool(name="hst", bufs=3) as hpool,
            tc.tile_pool(name="gath", bufs=3) as gpool,
            tc.tile_pool(name="amat", bufs=2) as apool,
            tc.tile_pool(name="work", bufs=4) as wpool,
            tc.tile_pool(name="ps0", bufs=2, space="PSUM") as ps0,
            tc.tile_pool(name="psag", bufs=4, space="PSUM") as psag,
        ):
            htab = dpool.tile([cfg.ntab, HC], bf16)
            accum = cpool.tile([P, nblk * DO], f32)
            nc.vector.memset(accum[:], 0.0)

            idxt = cpool.tile([P, tot // 16], i16)
            nc.sync.dma_start(out=idxt[:], in_=idx16[:])
            dct = cpool.tile([P, totcols], bf16)
            nc.sync.dma_start(out=dct[:], in_=dcolst[:])
            iotat = cpool.tile([P, P], bf16)
            nc.sync.dma_start(out=iotat[:], in_=iota[:])
            bbct = cpool.tile([P, DO], f32)
            nc.sync.dma_start(out=bbct[:], in_=bbc[:])
            wts = []
            for c in range(kchunks):
                wt = cpool.tile([P, DO], bf16, tag=f"w{c}")
                nc.sync.dma_start(out=wt[:], in_=wmat[c * P:(c + 1) * P, :])
                wts.append(wt)

            gsz_regs = [nc.alloc_register(mybir.EngineType.Pool, f"gsz{q}")
                        for q in range(4)]

            # ---- phase 0: build h-table, window-interleaved with calls ----
            win_tiles = []
            for g in range(ng):
                j0 = (g << cfg.gshift) // P
                j1 = min(((g + 1) << cfg.gshift), cfg.ntab) // P
                win_tiles.append((j0, j1))

            BT = 8   # FC tiles per batched DMA (sync sequencer is ~550ns/DMA)

            def build_window(g):
                j0, j1 = win_tiles[g]
                for b0 in range(j0, j1, BT):
                    nb = min(BT, j1 - b0)
                    ftb = ftpool.tile([P, BT * kchunks * P], bf16, tag="ft")
                    nc.sync.dma_start(
                        out=ftb[:, 0:nb * kchunks * P],
                        in_=ftt[:, b0 * kchunks * P:(b0 + nb) * kchunks * P])
                    for jj in range(nb):
                        j = b0 + jj
                        ph = ps0.tile([P, DO], f32, tag="ph")
                        for c in range(kchunks):
                            lo = (jj * kchunks + c) * P
                            nc.tensor.matmul(ph[:], lhsT=ftb[:, lo:lo + P],
                                             rhs=wts[c][:], start=(c == 0),
                                             stop=(c == kchunks - 1))
                        hs = hpool.tile([P, DO], bf16, tag="hs")
                        nc.vector.tensor_tensor(
                            out=hs[:], in0=ph[:], in1=bbct[:],
                            op=mybir.AluOpType.add)
                        # rows 256B-strided; cols 64:128 never read. Issue on
                        # the Act hwdge queue - the sync sequencer (~550ns/DMA)
                        # otherwise serializes window builds against the
                        # gather stream.
                        nc.scalar.dma_start(
                            out=htab[j * P:(j + 1) * P, 0:DO], in_=hs[:])

            # ---- main: calls in (g, s) order ------------------------------
            B_max = max(b for _, b in call_budget.values())
            for _ in range(3):
                gz = gpool.tile([P, (B_max // P) * HC], bf16, tag="gt",
                                name="gz")
                nc.vector.memset(gz[:], 0.0)

            built = set()
            call_no = 0
            for (s, g) in call_list:
                if g not in built:
                    build_window(g)
                    built.add(g)
                off, B = call_budget[(s, g)]
                glo = g << cfg.gshift
                ghi = min(glo + cfg.gsize, cfg.ntab)
                gt = gpool.tile([P, (B // P) * HC], bf16, tag="gt")
                gt3 = gt[:].rearrange("p (t e) -> p t e", e=HC)
                if os.environ.get("GCN_SKIP_GATHER"):
                    nc.vector.memset(gt[:, 0:1], 0.0)
                else:
                    q = call_no % 4
                    nc.gpsimd.reg_mov(gsz_regs[q], B)
                    nc.gpsimd.dma_gather(
                        out_ap=gt3,
                        in_ap=htab[glo:ghi, :],
                        idxs_ap=idxt[:, off // 16:(off + B) // 16],
                        num_idxs=B,
                        num_idxs_reg=gsz_regs[q],
                        elem_size=HC,
                        single_packet=False,
                        queue_num=q,
                    )
                call_no += 1

                # one-hot matrix for the whole call
                c0, ncols = meta["call_cols"][(s, g)]
                if ncols > 0:
                    ab = apool.tile([P, ncols * P], bf16, tag="ab")
                    d_b = dct[:, c0:c0 + ncols].to_broadcast([P, ncols, P])
                    iap = iotat[:]
                    i_b = bass.AP(iap.tensor, iap.offset,
                                  [iap.ap[0], [0, ncols], iap.ap[1]])
                    nc.vector.tensor_tensor(
                        out=ab[:].rearrange("p (t d) -> p t d", d=P),
                        in0=i_b, in1=d_b, op=mybir.AluOpType.is_equal)

                for k in supers[s]:
                    if (s, g, k) not in spans:
                        continue
                    t0, t1 = spans[(s, g, k)]
                    ps = psag.tile([P, DO], f32, tag="agg", name="aggps")
                    for t in range(t0, t1):
                        col = meta["col_index"][(s, g, k, t)]
                        amat = ab[:, (col - c0) * P:(col - c0 + 1) * P]
                        nc.tensor.matmul(ps[:], lhsT=amat,
                                         rhs=gt3[:, t, 0:DO],
                                         start=(t == t0), stop=(t == t1 - 1))
                    acc_k = accum[:, k * DO:(k + 1) * DO]
                    nc.vector.tensor_tensor(out=acc_k, in0=acc_k, in1=ps[:],
                                            op=mybir.AluOpType.add)
                    if meta["lastcall"][k] == (s, g):
                        rows = min(P, cfg.npc - k * P)
                        nc.sync.dma_start(out=out[k * P:k * P + rows, :],
                                          in_=acc_k[:rows])
    return nc


def _run_spmd(nc, in_maps, trace=False):
    from concourse.bass_utils import run_bass_kernel_spmd
    return run_bass_kernel_spmd(nc, in_maps, list(range(len(in_maps))),
                                trace=trace)


_PROGRAM_CACHE = {}


def gcn_kernel(feature, W, b, src, dst, cfg=FULL_CFG, trace=False):
    in_maps, meta = _prep_host(feature, W, b, src, dst, cfg)
    key = (cfg.N, meta["tot"], meta["totcols"],
           tuple(v for _, v in sorted(meta["call_budget"].items())),
           tuple(sorted(meta["col_index"].keys())))
    nc = _PROGRAM_CACHE.get(key)
    if nc is None:
        nc = _build_program(cfg, meta)
        nc.finalize()
        _PROGRAM_CACHE[key] = nc
    res = _run_spmd(nc, in_maps, trace=trace)
    outs = [res.results[m]["out"] for m in range(cfg.ncores)]
    full = np.concatenate(outs, axis=0).astype(np.float32)
    return full, res


def kernel(**inputs):
    feature = np.asarray(inputs["feature"], dtype=np.float32)
    W = np.asarray(inputs["W"], dtype=np.float32)
    b = np.asarray(inputs["b"], dtype=np.float32)
    src = np.asarray(inputs["src"], dtype=np.int32)
    dst = np.asarray(inputs["dst"], dtype=np.int32)
    full, _ = gcn_kernel(feature, W, b, src, dst, FULL_CFG)
    return full


# revision 24
# speedup vs baseline: 1.5409x; 1.0873x over previous
"""GCN layer (fc + gather/scatter-sum) on 8 trn2 NeuronCores.

v2: FC-first h-table + compact SWDGE gather.

  h = feature @ W + b   (built on-device, per core, into a DRAM table of
                         bf16 rows padded to 128 cols = 256B — the minimum
                         SWDGE gather element)
  out = segment_sum(take(h, src), dst)   (dma_gather of 256B rows + one-hot
                         matmul aggregation per 128-dst block)

The desc-gen ucode (one Q7 pair, ~3.5ns/desc serialized on GpSimd) is the
wall, so descriptors are minimized: edges are packed COMPACTLY per call
(no per-(block,group) tile rounding) and per-core trailing -1 indices let
the ucode trim to each core's actual edge count. Tiles may straddle block
boundaries; the one-hot dst matrix is built from host-shipped per-(block,
tile) columns (foreign/pad slots = -1, so is_equal never matches).
Block PSUM accumulators live across all 4 source-window calls; blocks are
processed in two phases (52+46) to fit PSUM. The h-table is a DRAM tile,
so Tile's subtile deps order window builds before their gathers, letting
phase-0 FC overlap the first window's gather stream.
"""

import os
import numpy as np
import ml_dtypes

import concourse.bass as bass
import concourse.bacc as bacc
import concourse.mybir as mybir
from concourse import tile
from concourse import library_config

P = 128
NCORES = 8


def _patch_tile_exit():
    """The walrus build in this container rejects two constructs Tile emits
    at TileContext exit: a Drain carrying more than one sync wait ("Too many
    sync wait commands") and the sem_clear InstISA ("ISA wrong length").
    Replace the exit sequence with equivalent one-wait-per-Drain chains and
    skip the semaphore clears (fine for single-execution NEFFs)."""
    import bass_rust
    from concourse.vector_clock import ScopedClock

    def _drain_and_barrier(self, tick_clock, wait_clock):
        drain_inst = self.nc.sync.drain()
        wait_clock.add_sem_waits(
            drain_inst.ins, ScopedClock({None: tick_clock.global_clock})
        )
        si = drain_inst.ins.sync_info
        if si is not None and len(si.on_wait) > 1:
            waits = list(si.on_wait)
            drain_inst.ins.sync_info = bass_rust.SyncInfo(
                on_wait=waits[:1], on_update=list(si.on_update))
            for w in waits[1:]:
                extra = self.nc.sync.drain()
                extra.ins.sync_info = bass_rust.SyncInfo(
                    on_wait=[w], on_update=[])
        self.nc.all_engine_barrier()
        popped = self.nc._tile_sem_poison_stack.pop()
        assert popped is self._sem_poison
        self.nc.all_engine_barrier()

    tile.TileContext._drain_and_barrier = _drain_and_barrier


_patch_tile_exit()


class Cfg:
    def __init__(self, n_nodes, d_in, d_out, ncores, group_shift, sb_blocks,
                 psum_blocks):
        self.N = n_nodes
        self.D = d_in
        self.DO = d_out
        self.ncores = ncores
        self.gshift = group_shift          # src window = [g << gshift, (g+1) << gshift)
        self.gsize = 1 << group_shift
        self.ngroups = (n_nodes + self.gsize - 1) >> group_shift
        self.npc = n_nodes // ncores       # nodes per core
        self.nblk = (self.npc + P - 1) // P
        self.sb = sb_blocks                # blocks per superbatch (= per call)
        self.psb = psum_blocks             # max concurrent block accumulators
        self.ntab = ((n_nodes + P - 1) // P) * P  # padded table rows


FULL_CFG = Cfg(n_nodes=100000, d_in=256, d_out=64, ncores=8, group_shift=15,
               sb_blocks=13, psum_blocks=52)


def _prep_host(feature, W, b, src, dst, cfg):
    """Shard + sort edges; build compact per-call slot layout and the
    per-(block, tile) one-hot dst columns."""
    N, npc, nblk, ng = cfg.N, cfg.npc, cfg.nblk, cfg.ngroups
    src = np.asarray(src, dtype=np.int64)
    dst = np.asarray(dst, dtype=np.int64)

    nsb = (nblk + cfg.sb - 1) // cfg.sb
    supers = [range(s * cfg.sb, min((s + 1) * cfg.sb, nblk)) for s in range(nsb)]

    # per-core sorted edge lists per (super, group, block)
    per_core = []   # m -> {(s, g, k): (es, ed_rel)}
    counts = np.zeros((cfg.ncores, nsb, ng, nblk), dtype=np.int64)
    for m in range(cfg.ncores):
        lo, hi = m * npc, (m + 1) * npc
        mask = (dst >= lo) & (dst < hi)
        es = src[mask]
        ed = dst[mask] - lo
        blk = ed >> 7
        grp = es >> cfg.gshift
        sup = blk // cfg.sb
        order = np.lexsort((es, blk, grp, sup))
        es, ed, blk, grp, sup = (a[order] for a in (es, ed, blk, grp, sup))
        np.add.at(counts[m], (sup, grp, blk), 1)
        key = sup * (ng * nblk) + grp * nblk + blk
        bounds = np.searchsorted(key, np.arange(nsb * ng * nblk + 1))
        d = {}
        for s in range(nsb):
            for g in range(ng):
                for k in supers[s]:
                    a0 = bounds[s * ng * nblk + g * nblk + k]
                    a1 = bounds[s * ng * nblk + g * nblk + k + 1]
                    if a1 > a0:
                        d[(s, g, k)] = (es[a0:a1], ed[a0:a1])
        per_core.append(d)

    # call order: g -> s.  Compact slot layout per call; shared call
    # budget = roundup(max_m total, P).  Shared per-(k) tile spans =
    # union over cores of [start/P, ceil(end/P)).
    call_list = []          # (s, g) in call order
    call_budget = {}        # (s, g) -> (slot_offset, B)
    spans = {}              # (s, g, k) -> (t0, t1) tile span within call
    core_starts = {}        # (m, s, g, k) -> slot start within call
    pos = 0
    for g in range(ng):
        for s in range(nsb):
            tot_m = counts[:, s, g, :].sum(axis=1)
            B = int(np.ceil(max(int(tot_m.max()), 1) / P)) * P
            call_list.append((s, g))
            call_budget[(s, g)] = (pos, B)
            t0s = {k: 10 ** 9 for k in supers[s]}
            t1s = {k: -1 for k in supers[s]}
            for m in range(cfg.ncores):
                off = 0
                for k in supers[s]:
                    n = int(counts[m, s, g, k])
                    core_starts[(m, s, g, k)] = off
                    if n > 0:
                        t0s[k] = min(t0s[k], off // P)
                        t1s[k] = max(t1s[k], (off + n - 1) // P)
                    off += n
            for k in supers[s]:
                if t1s[k] >= 0:
                    spans[(s, g, k)] = (t0s[k], t1s[k] + 1)
            pos += B
    tot = pos
    assert tot % P == 0

    # column layout: per call, for k in super, for t in span(k): one column
    col_index = {}          # (s, g, k, t) -> global column index
    call_cols = {}          # (s, g) -> (col0, ncols)
    cpos = 0
    for (s, g) in call_list:
        c0 = cpos
        for k in supers[s]:
            if (s, g, k) not in spans:
                continue
            t0, t1 = spans[(s, g, k)]
            for t in range(t0, t1):
                col_index[(s, g, k, t)] = cpos
                cpos += 1
        call_cols[(s, g)] = (c0, cpos - c0)
    totcols = max(cpos, 1)

    # per-(k) global matmul sequence (for start/stop flags)
    kseq = {k: [] for k in range(nblk)}
    for (s, g) in call_list:
        for k in supers[s]:
            if (s, g, k) not in spans:
                continue
            t0, t1 = spans[(s, g, k)]
            for t in range(t0, t1):
                kseq[k].append((s, g, t, col_index[(s, g, k, t)]))
    for k in range(nblk):
        assert kseq[k], f"block {k} has no edges on some core"

    # host arrays
    ftab = np.ascontiguousarray(feature.astype(ml_dtypes.bfloat16))
    ntiles = cfg.ntab // P
    kchunks = cfg.D // P
    # partition-major FC operand: ftt[p, ((j*kchunks+c)*P)+n] =
    #   feature[j*P+n, c*P+p]  (so ft[:, (j*kchunks+c)*P:+P] is lhsT)
    fpad = np.zeros((cfg.ntab, cfg.D), dtype=ml_dtypes.bfloat16)
    fpad[:N] = ftab
    ftt = np.ascontiguousarray(
        fpad.reshape(ntiles, P, kchunks, P).transpose(3, 0, 2, 1)
        .reshape(P, ntiles * kchunks * P))
    wmat = np.ascontiguousarray(W.astype(ml_dtypes.bfloat16))
    bbc = np.ascontiguousarray(
        np.tile(b.astype(np.float32)[None, :], (P, 1)))
    iota = np.ascontiguousarray(
        np.tile(np.arange(P, dtype=np.float32)[None, :], (P, 1)).astype(
            ml_dtypes.bfloat16))

    # pad slots gather row 0 of the window (a -1/trailing-trim variant
    # crashed the device; zero-pad costs ~2% extra descriptors)
    in_maps = []
    for m in range(cfg.ncores):
        d = per_core[m]
        idx_arr = np.zeros(tot, dtype=np.int16)
        dslot = np.full(tot, -1.0, dtype=np.float32)   # dst-local per slot
        kown = np.full(tot, -2, dtype=np.int64)        # owning block per slot
        for (s, g) in call_list:
            off, B = call_budget[(s, g)]
            for k in supers[s]:
                if (s, g, k) not in d:
                    continue
                es, ed = d[(s, g, k)]
                a = core_starts[(m, s, g, k)]
                n = len(es)
                idx_arr[off + a:off + a + n] = (
                    es - (g << cfg.gshift)).astype(np.int16)
                dslot[off + a:off + a + n] = (ed - k * P).astype(np.float32)
                kown[off + a:off + a + n] = k
        # per-(k,t) one-hot columns: partition p of column (s,g,k,t) =
        # dst-local of slot off+t*P+p if that slot belongs to block k else -1
        dcols = np.full((P, totcols), -1.0, dtype=np.float32)
        for (s, g, k, t), col in col_index.items():
            off, B = call_budget[(s, g)]
            sl = slice(off + t * P, off + t * P + P)
            dcols[:, col] = np.where(kown[sl] == k, dslot[sl], -1.0)
        idx16 = np.ascontiguousarray(
            np.tile(idx_arr.reshape(tot // 16, 16).T, (P // 16, 1)))
        dcolst = np.ascontiguousarray(dcols.astype(ml_dtypes.bfloat16))
        in_maps.append({
            "ftt": ftt, "wmat": wmat, "bbc": bbc, "iota": iota,
            "idx16": idx16, "dcolst": dcolst,
        })

    lastcall = {k: (kseq[k][-1][0], kseq[k][-1][1]) for k in range(nblk)}
    meta = dict(call_list=call_list, call_budget=call_budget, spans=spans,
                kseq=kseq, supers=supers, tot=tot, totcols=totcols,
                call_cols=call_cols, col_index=col_index, lastcall=lastcall)
    return in_maps, meta


def _build_program(cfg, meta):
    N, D, DO, nblk, ng = cfg.N, cfg.D, cfg.DO, cfg.nblk, cfg.ngroups
    call_list, call_budget = meta["call_list"], meta["call_budget"]
    spans, kseq, supers = meta["spans"], meta["kseq"], meta["supers"]
    tot, totcols = meta["tot"], meta["totcols"]
    bf16, f32, i16 = mybir.dt.bfloat16, mybir.dt.float32, mybir.dt.int16
    ntiles = cfg.ntab // P
    kchunks = D // P

    nc = bacc.Bacc(None, target_bir_lowering=False, num_swdge_queues=4)
    ftt = nc.dram_tensor("ftt", [P, ntiles * kchunks * P], bf16,
                         kind="ExternalInput")
    wmat = nc.dram_tensor("wmat", [D, DO], bf16, kind="ExternalInput")
    bbc = nc.dram_tensor("bbc", [P, DO], f32, kind="ExternalInput")
    iota = nc.dram_tensor("iota", [P, P], bf16, kind="ExternalInput")
    idx16 = nc.dram_tensor("idx16", [P, tot // 16], i16, kind="ExternalInput")
    dcolst = nc.dram_tensor("dcolst", [P, totcols], bf16, kind="ExternalInput")
    out = nc.dram_tensor("out", [cfg.npc, DO], f32, kind="ExternalOutput")

    # h-table rows padded to 128 cols (256B) for the SWDGE elem minimum
    HC = 2 * DO

    with tile.TileContext(nc) as tc:
        with (
            tc.tile_pool(name="const", bufs=1) as cpool,
            tc.tile_pool(name="dram", bufs=1, space="DRAM") as dpool,
            tc.tile_pool(name="ftp", bufs=3) as ftpool,
            tc.tile_pool(name="hst", bufs=3) as hpool,
            tc.tile_pool(name="gath", bufs=4) as gpool,
            tc.tile_pool(name="amat", bufs=2) as apool,
            tc.tile_pool(name="work", bufs=4) as wpool,
            tc.tile_pool(name="ps0", bufs=2, space="PSUM") as ps0,
            tc.tile_pool(name="psag", bufs=4, space="PSUM") as psag,
        ):
            htab = dpool.tile([cfg.ntab, HC], bf16)
            accum = cpool.tile([P, nblk * DO], f32)
            nc.vector.memset(accum[:], 0.0)

            idxt = cpool.tile([P, tot // 16], i16)
            nc.sync.dma_start(out=idxt[:], in_=idx16[:])
            dct = cpool.tile([P, totcols], bf16)
            nc.sync.dma_start(out=dct[:], in_=dcolst[:])
            iotat = cpool.tile([P, P], bf16)
            nc.sync.dma_start(out=iotat[:], in_=iota[:])
            bbct = cpool.tile([P, DO], f32)
            nc.sync.dma_start(out=bbct[:], in_=bbc[:])
            wts = []
            for c in range(kchunks):
                wt = cpool.tile([P, DO], bf16, tag=f"w{c}")
                nc.sync.dma_start(out=wt[:], in_=wmat[c * P:(c + 1) * P, :])
                wts.append(wt)

            gsz_regs = [nc.alloc_register(mybir.EngineType.Pool, f"gsz{q}")
                        for q in range(4)]

            # ---- phase 0: build h-table, window-interleaved with calls ----
            win_tiles = []
            for g in range(ng):
                j0 = (g << cfg.gshift) // P
                j1 = min(((g + 1) << cfg.gshift), cfg.ntab) // P
                win_tiles.append((j0, j1))

            BT = 8   # FC tiles per batched DMA (sync sequencer is ~550ns/DMA)

            def build_window(g):
                j0, j1 = win_tiles[g]
                for b0 in range(j0, j1, BT):
                    nb = min(BT, j1 - b0)
                    ftb = ftpool.tile([P, BT * kchunks * P], bf16, tag="ft")
                    nc.sync.dma_start(
                        out=ftb[:, 0:nb * kchunks * P],
                        in_=ftt[:, b0 * kchunks * P:(b0 + nb) * kchunks * P])
                    for jj in range(nb):
                        j = b0 + jj
                        ph = ps0.tile([P, DO], f32, tag="ph")
                        for c in range(kchunks):
                            lo = (jj * kchunks + c) * P
                            nc.tensor.matmul(ph[:], lhsT=ftb[:, lo:lo + P],
                                             rhs=wts[c][:], start=(c == 0),
                                             stop=(c == kchunks - 1))
                        hs = hpool.tile([P, DO], bf16, tag="hs")
                        nc.vector.tensor_tensor(
                            out=hs[:], in0=ph[:], in1=bbct[:],
                            op=mybir.AluOpType.add)
                        # rows 256B-strided; cols 64:128 never read. Issue on
                        # the Act hwdge queue - the sync sequencer (~550ns/DMA)
                        # otherwise serializes window builds against the
                        # gather stream.
                        nc.scalar.dma_start(
                            out=htab[j * P:(j + 1) * P, 0:DO], in_=hs[:])

            # ---- main: calls in (g, s) order ------------------------------
            B_max = max(b for _, b in call_budget.values())
            for _ in range(4):
                gz = gpool.tile([P, (B_max // P) * HC], bf16, tag="gt",
                                name="gz")
                nc.vector.memset(gz[:], 0.0)

            # emit ALL window builds up front: the static per-engine
            # schedule otherwise parks window g+1's FC matmuls behind
            # window g's agg matmuls on the PE stream (the scheduler's
            # desc-gen cost model is ~10x optimistic), stalling each
            # window hand-off ~280us
            for g in range(ng):
                build_window(g)

            call_no = 0
            for (s, g) in call_list:
                off, B = call_budget[(s, g)]
                glo = g << cfg.gshift
                ghi = min(glo + cfg.gsize, cfg.ntab)
                gt = gpool.tile([P, (B // P) * HC], bf16, tag="gt")
                gt3 = gt[:].rearrange("p (t e) -> p t e", e=HC)
                if os.environ.get("GCN_SKIP_GATHER"):
                    nc.vector.memset(gt[:, 0:1], 0.0)
                else:
                    q = call_no % 4
                    nc.gpsimd.reg_mov(gsz_regs[q], B)
                    nc.gpsimd.dma_gather(
                        out_ap=gt3,
                        in_ap=htab[glo:ghi, :],
                        idxs_ap=idxt[:, off // 16:(off + B) // 16],
                        num_idxs=B,
                        num_idxs_reg=gsz_regs[q],
                        elem_size=HC,
                        single_packet=False,
                        queue_num=q,
                    )
                call_no += 1

                # one-hot matrix for the whole call
                c0, ncols = meta["call_cols"][(s, g)]
                if ncols > 0:
                    ab = apool.tile([P, ncols * P], bf16, tag="ab")
                    d_b = dct[:, c0:c0 + ncols].to_broadcast([P, ncols, P])
                    iap = iotat[:]
                    i_b = bass.AP(iap.tensor, iap.offset,
                                  [iap.ap[0], [0, ncols], iap.ap[1]])
                    nc.vector.tensor_tensor(
                        out=ab[:].rearrange("p (t d) -> p t d", d=P),
                        in0=i_b, in1=d_b, op=mybir.AluOpType.is_equal)

                for k in supers[s]:
                    if (s, g, k) not in spans:
                        continue
                    t0, t1 = spans[(s, g, k)]
                    ps = psag.tile([P, DO], f32, tag="agg", name="aggps")
                    for t in range(t0, t1):
                        col = meta["col_index"][(s, g, k, t)]
                        amat = ab[:, (col - c0) * P:(col - c0 + 1) * P]
                        nc.tensor.matmul(ps[:], lhsT=amat,
                                         rhs=gt3[:, t, 0:DO],
                                         start=(t == t0), stop=(t == t1 - 1))
                    acc_k = accum[:, k * DO:(k + 1) * DO]
                    nc.vector.tensor_tensor(out=acc_k, in0=acc_k, in1=ps[:],
                                            op=mybir.AluOpType.add)
                    if meta["lastcall"][k] == (s, g):
                        rows = min(P, cfg.npc - k * P)
                        nc.sync.dma_start(out=out[k * P:k * P + rows, :],
                                          in_=acc_k[:rows])
    return nc


def _run_spmd(nc, in_maps, trace=False):
    from concourse.bass_utils import run_bass_kernel_spmd
    return run_bass_kernel_spmd(nc, in_maps, list(range(len(in_maps))),
                                trace=trace)


_PROGRAM_CACHE = {}


def gcn_kernel(feature, W, b, src, dst, cfg=FULL_CFG, trace=False):
    in_maps, meta = _prep_host(feature, W, b, src, dst, cfg)
    key = (cfg.N, meta["tot"], meta["totcols"],
           tuple(v for _, v in sorted(meta["call_budget"].items())),
           tuple(sorted(meta["col_index"].keys())))
    nc = _PROGRAM_CACHE.get(key)
    if nc is None:
        nc = _build_program(cfg, meta)
        nc.finalize()
        _PROGRAM_CACHE[key] = nc
    res = _run_spmd(nc, in_maps, trace=trace)
    outs = [res.results[m]["out"] for m in range(cfg.ncores)]
    full = np.concatenate(outs, axis=0).astype(np.float32)
    return full, res


def kernel(**inputs):
    feature = np.asarray(inputs["feature"], dtype=np.float32)
    W = np.asarray(inputs["W"], dtype=np.float32)
    b = np.asarray(inputs["b"], dtype=np.float32)
    src = np.asarray(inputs["src"], dtype=np.int32)
    dst = np.asarray(inputs["dst"], dtype=np.int32)
    full, _ = gcn_kernel(feature, W, b, src, dst, FULL_CFG)
    return full
